# revision 6
# baseline (speedup 1.0000x reference)
"""NeRF-small MLP Bass kernel for Trainium2, 8-core data parallel.

Hidden-on-partitions, points-on-free-dim layout: input x[N,6] is loaded
point-major, repacked 6->8 slots to bf16, and xbar-transposed to
t4[128,512] where row 8q+c = channel c of 512-point slab q. All layers run
as K<=128, M<=128, N=512 matmuls with block-diagonal weights processing two
slabs (one "pair") per pass.

- geo_feat is linear in h1 and only feeds c0, so ws2[:,1:]@wc0[3:] is folded
  into one 64x64 matrix on the host: 8 matmuls and 5 PSUM->SBUF relu moves
  per 1024 points.
- rgb + sigma accumulate across all 8 pairs of a super-tile into one [64,512]
  PSUM bank whose rows are already output-ordered (row 4q+c'), so a single
  copy stages the output per super-tile.
- Each pair's whole 5-stage layer chain lives in ONE rotating PSUM bank: a
  stage's matmul already waits on the previous stage's relu (data dep), so
  the same-bank write-after-read costs nothing. 7 rotating banks -> up to 7
  pairs in flight, which keeps the PE busy through every relu latency.
- Relu moves alternate Act/DVE (21:19, Act also owns the output-stage copy,
  DVE the softplus arithmetic); softplus is relu(x) + poly-ln1p(exp(-|x|))
  so the ACT engine only ever needs one function-table set (no reloads).
- Both 4-chunk transposes per super-tile are single 3D-AP xbar instructions;
  input copies are paced behind the previous transpose (add_dep_helper) so
  the xbar mode-switch serialization never stalls a transpose; the
  post-transpose epilogue half is deferred one super-tile so its ops never
  head-of-line block the relu streams.

Point permutation within a 8192-pt super-tile: pt(k,p,q) = base + 2048k +
16p + q with t4 row 8q+c, col 128k+p - identical on input and output paths.
"""

import numpy as np
import ml_dtypes

N_TOTAL = 1048576
N_CORES = 8
NPC = N_TOTAL // N_CORES       # 131072 points per core
ST = 8192                      # points per super-tile
N_ST = NPC // ST

_CACHE = {}


def _pack_weights(ws0, ws1, ws2, wc0, wc1, wc2, wc3):
    """Build block-diagonal 'big' stationary matrices (see module docstring)."""
    bf16 = ml_dtypes.bfloat16
    f32 = np.float32
    ws0, ws1, ws2, wc0, wc1, wc2, wc3 = [
        np.asarray(w, f32) for w in (ws0, ws1, ws2, wc0, wc1, wc2, wc3)
    ]
    w0big = np.zeros((128, 8 * 128), f32)
    c0vbig = np.zeros((128, 8 * 128), f32)
    for j in range(8):
        for q, off in ((2 * j, 0), (2 * j + 1, 64)):
            w0big[8 * q: 8 * q + 3, 128 * j + off: 128 * j + off + 64] = ws0
            c0vbig[8 * q + 3: 8 * q + 6, 128 * j + off: 128 * j + off + 64] = wc0[0:3]

    def blockdiag(m):
        b = np.zeros((128, 128), f32)
        b[0:64, 0:64] = m
        b[64:128, 64:128] = m
        return b

    s1big = blockdiag(ws1)
    c0mbig = blockdiag(ws2[:, 1:16] @ wc0[3:18])  # geo folded into c0
    c1big = blockdiag(wc1)
    c2big = blockdiag(wc2)
    # Final layer: pair j scatters rgb into rows 8j+{0..2}, 8j+{4..6} and
    # sigma_raw into rows 8j+3, 8j+7 of the shared [64,512] bank (= row 4q+c').
    c3big = np.zeros((128, 8 * 64), f32)
    sigbig = np.zeros((128, 8 * 64), f32)
    for j in range(8):
        base = 64 * j + 8 * j
        c3big[0:64, base: base + 3] = wc3[:, 0:3]
        c3big[64:128, base + 4: base + 7] = wc3[:, 0:3]
        sigbig[0:64, base + 3] = ws2[:, 0]
        sigbig[64:128, base + 7] = ws2[:, 0]
    # single packed tensor; first-needed weights (w0big, s1big) lead so a
    # small head DMA unblocks the first matmuls while the rest transfers
    wpack = np.concatenate(
        [w0big, s1big, c0vbig, c0mbig, c1big, c2big, c3big, sigbig],
        axis=1)
    return {"wpack": wpack.astype(bf16)}


def _build(npts):
    import concourse.mybir as mybir
    from concourse import bacc, tile
    from concourse.tile import add_dep_helper

    dt = mybir.dt
    f32, bf16 = dt.float32, dt.bfloat16
    AF = mybir.ActivationFunctionType

    nc = bacc.Bacc()
    x_in = nc.dram_tensor("x", [npts, 6], f32, kind="ExternalInput")
    out = nc.dram_tensor("out", [npts, 4], f32, kind="ExternalOutput")
    wcols = [("w0big", 1024), ("s1big", 128), ("c0vbig", 1024),
             ("c0mbig", 128), ("c1big", 128), ("c2big", 128),
             ("c3big", 512), ("sigbig", 512)]
    wtot = sum(c for _, c in wcols)
    wdr = nc.dram_tensor("wpack", [128, wtot], bf16, kind="ExternalInput")

    n_st = npts // ST
    # x viewed per super-tile: partition p, free (k chunk, q slot, c channel)
    xv = x_in.rearrange("(s k p q) c -> s p k (q c)", k=4, p=128, q=16)
    ov = out.rearrange("(s k p q) c -> s p k (q c)", k=4, p=128, q=16)

    with tile.TileContext(nc) as tc:
        with (
            tc.tile_pool(name="const", bufs=1) as constp,
            tc.tile_pool(name="xin", bufs=4) as xinp,
            tc.tile_pool(name="t4", bufs=4) as t4p,
            tc.tile_pool(name="act", bufs=6) as actp,
            tc.tile_pool(name="h1", bufs=10) as h1p,
            tc.tile_pool(name="ost", bufs=3) as ostp,
            tc.tile_pool(name="opm", bufs=3) as opmp,
            tc.tile_pool(name="psL", bufs=7, space="PSUM") as psLp,
            tc.tile_pool(name="psC", bufs=1, space="PSUM") as psCp,
        ):
            wt = constp.tile([128, wtot], bf16, tag="wpack")
            W = {}
            off = 0
            for name, cols in wcols:
                W[name] = wt[:, off: off + cols]
                off += cols

            # three persistent repack buffers (rotate across super-tiles);
            # pad slots (q,6) (q,7) zeroed once so zero weight rows see
            # finite data
            x8s = []
            for i in range(3):
                t = constp.tile([128, 512], bf16, tag=f"x8_{i}")
                nc.vector.memset(t[:], 0.0)
                x8s.append(t)

            # relu engine pattern per super-tile: adjacent stages alternate
            # Act/DVE; 21:19 split because DVE carries the sigma epilogue
            RELU_PAT = "ADADA" * 5 + "DADAD" * 3
            rcnt = [0]

            def relu(dst, src):
                if RELU_PAT[rcnt[0] % 40] == "A":
                    nc.scalar.activation(dst, src, AF.Relu)
                else:
                    nc.vector.tensor_scalar_max(dst, src, 0.0)
                rcnt[0] += 1

            prev_tin = [None]

            def input_path(s):
                xin = xinp.tile([128, 384], f32, tag="xin")
                xi = nc.sync.dma_start(
                    xin.rearrange("p (k m) -> p k m", k=4), xv[s])
                if prev_tin[0] is not None:
                    # keep future input copies from being scheduler-hoisted
                    # ahead of earlier transposes: the xbar mode-switch
                    # serialization would make those transposes wait on them
                    add_dep_helper(xi.ins, prev_tin[0].ins,
                                   reason="xbar: xin copy after previous tin")
                x8 = x8s[s % 3]
                nc.gpsimd.tensor_copy(
                    x8.rearrange("p (k q c) -> p k q c", k=4, c=8)[:, :, :, 0:6],
                    xin.rearrange("p (k q c) -> p k q c", k=4, c=6),
                )
                t4 = t4p.tile([128, 512], bf16, tag="t4")
                # 3D out AP = 4 independent 128x128 chunk transposes in ONE
                # xbar instruction (one HWDGE pass instead of 4 serialized)
                prev_tin[0] = nc.sync.dma_start_transpose(
                    t4.rearrange("r (k p) -> r k p", k=4), x8[:])
                return t4

            def epilogue_post(s, opm, fast=False):
                # fast=True for the final super-tile: it is pure tail latency
                # after the compute drain, so run on the faster DVE/SP engines
                ew = nc.vector if fast else nc.gpsimd
                of32 = opmp.tile([128, 256], f32, tag="of32")
                ew.tensor_copy(of32[:], opm[:])
                # sigma slots carry raw values; softplus as
                # relu(x) + ln1p(exp(-|x|)) with ln1p a degree-6 polynomial
                # (max err 3e-6 on [0,1]) so the ACT engine only ever needs
                # Relu/Copy/Exp - all in one table set, zero table reloads.
                opm3 = opm.rearrange("p (m c) -> p m c", c=4)[:, :, 3]
                of3 = of32.rearrange("p (m c) -> p m c", c=4)[:, :, 3]
                spm = opmp.tile([128, 64], f32, tag="spm")
                spu = opmp.tile([128, 64], f32, tag="spu")
                spt = opmp.tile([128, 64], f32, tag="spt")
                # arithmetic stays on DVE: TensorScalar/ScalarTensorTensor
                # opcodes are not legal on the Pool (GpSimd) engine.
                # ln1p(u) ~ u(a1+u(a2+u(a3+u*a4))), max err 1.3e-4 on [0,1];
                # final step fuses relu(x)+P via (x max 0) add P
                ALU = mybir.AluOpType
                nc.vector.scalar_tensor_tensor(
                    spm[:], opm3, -1.0, opm3, op0=ALU.mult, op1=ALU.min)
                nc.scalar.activation(spu[:], spm[:], AF.Exp)
                LN1P = [0.99712544, -0.47001579, 0.22433453, -0.0584286]
                nc.vector.tensor_scalar_mul(spt[:], spu[:], LN1P[3])
                for a in (LN1P[2], LN1P[1], LN1P[0]):
                    nc.vector.scalar_tensor_tensor(
                        spt[:], spt[:], a, spu[:], op0=ALU.add, op1=ALU.mult)
                nc.vector.scalar_tensor_tensor(
                    of3, opm3, 0.0, spt[:], op0=ALU.max, op1=ALU.add)
                (nc.sync if fast else nc.gpsimd).dma_start(
                    ov[s], of32.rearrange("p (k m) -> p k m", k=4))

            # input prefetched two super-tiles ahead so the in-order SP/Pool
            # queues never trap it behind epilogue work
            from collections import deque
            pending_post = deque()
            t4q = deque([input_path(0)])
            # weights follow the first input path so xin(0) heads the DMA
            # queue and tin(0)'s xbar mode-switch wait clears early; the
            # small head DMA covers the first matmuls' weights
            nc.scalar.dma_start(wt[:, 0:1152], wdr[:, 0:1152])
            wb = nc.scalar.dma_start(wt[:, 1152:wtot], wdr[:, 1152:wtot])
            add_dep_helper(wb.ins, prev_tin[0].ins,
                           reason="xbar: bulk weights after first tin")
            t4q.append(input_path(1))
            for s in range(n_st):
                if s == n_st - 1 and pending_post:
                    # last body: run the previous tile's post first so the
                    # final tout isn't xbar-blocked by its out-copy
                    epilogue_post(*pending_post.popleft())
                if s + 2 < n_st:
                    t4q.append(input_path(s + 2))
                t4 = t4q.popleft()
                C3 = psCp.tile([64, 512], f32, tag="C3")
                for j in range(8):
                    # one PSUM bank per pair: every stage's matmul waits on the
                    # previous stage's relu anyway, so in-place reuse is free
                    P = psLp.tile([128, 512], f32, tag="L")

                    nc.tensor.matmul(P[:], W["w0big"][:, 128 * j: 128 * (j + 1)],
                                     t4[:], start=True, stop=True)
                    h0 = actp.tile([128, 512], bf16, tag="h0")
                    relu(h0[:], P[:])

                    nc.tensor.matmul(P[:], W["s1big"][:], h0[:], start=True, stop=True)
                    h1 = h1p.tile([128, 512], bf16, tag="h1")
                    relu(h1[:], P[:])

                    nc.tensor.matmul(P[:], W["c0vbig"][:, 128 * j: 128 * (j + 1)],
                                     t4[:], start=True, stop=False)
                    nc.tensor.matmul(P[:], W["c0mbig"][:], h1[:], start=False, stop=True)
                    c0 = actp.tile([128, 512], bf16, tag="c0")
                    relu(c0[:], P[:])

                    nc.tensor.matmul(P[:], W["c1big"][:], c0[:], start=True, stop=True)
                    c1h = actp.tile([128, 512], bf16, tag="c1h")
                    relu(c1h[:], P[:])

                    nc.tensor.matmul(P[:], W["c2big"][:], c1h[:], start=True, stop=True)
                    c2h = actp.tile([128, 512], bf16, tag="c2h")
                    relu(c2h[:], P[:])

                    nc.tensor.matmul(C3[:], W["c3big"][:, 64 * j: 64 * (j + 1)],
                                     c2h[:], start=(j == 0), stop=False)
                    nc.tensor.matmul(C3[:], W["sigbig"][:, 64 * j: 64 * (j + 1)],
                                     h1[:], start=False, stop=(j == 7))

                O = ostp.tile([64, 512], bf16, tag="O")
                nc.scalar.activation(O[:], C3[:], AF.Copy)

                opm = opmp.tile([128, 256], bf16, tag="opm")
                nc.sync.dma_start_transpose(
                    opm.rearrange("p (k r) -> p k r", k=4), O[:])
                # the post-transpose half of the epilogue is deferred one
                # super-tile (emitted as epilogue_post(s-1) during body s):
                # when those ops reach their engine-queue heads the xbar
                # round-trip is long finished, so nothing head-of-line
                # blocks the relu streams
                if s == n_st - 1:
                    pending_post.append((s, opm))
                    epilogue_post(*pending_post.popleft(), fast=True)
                else:
                    pending_post.append((s, opm))
                    if s > 0:
                        epilogue_post(*pending_post.popleft())
    nc.compile()
    return nc


def _run(inputs, npts=NPC, trace=False, cores=N_CORES):
    from concourse import bass_utils

    key = npts
    if key not in _CACHE:
        _CACHE[key] = _build(npts)
    nc = _CACHE[key]
    wm = _pack_weights(inputs["ws0"], inputs["ws1"], inputs["ws2"],
                       inputs["wc0"], inputs["wc1"], inputs["wc2"], inputs["wc3"])
    x = np.ascontiguousarray(np.asarray(inputs["x"], np.float32))
    xs = x.reshape(cores, npts, 6)
    in_maps = [dict(wm, x=np.ascontiguousarray(xs[c])) for c in range(cores)]
    res = bass_utils.run_bass_kernel_spmd(
        nc, in_maps, core_ids=list(range(cores)), trace=trace)
    outs = np.concatenate([r["out"] for r in res.results], axis=0)
    return outs, res


def kernel(**inputs):
    out, _ = _run(inputs)
    return out.astype(np.float32)


# revision 7
# speedup vs baseline: 1.0015x; 1.0015x over previous
"""NeRF-small MLP Bass kernel for Trainium2, 8-core data parallel.

Hidden-on-partitions, points-on-free-dim layout: input x[N,6] is loaded
point-major, repacked 6->8 slots to bf16, and xbar-transposed to
t4[128,512] where row 8q+c = channel c of 512-point slab q. All layers run
as K<=128, M<=128, N=512 matmuls with block-diagonal weights processing two
slabs (one "pair") per pass.

- geo_feat is linear in h1 and only feeds c0, so ws2[:,1:]@wc0[3:] is folded
  into one 64x64 matrix on the host: 8 matmuls and 5 PSUM->SBUF relu moves
  per 1024 points.
- rgb + sigma accumulate across all 8 pairs of a super-tile into one [64,512]
  PSUM bank whose rows are already output-ordered (row 4q+c'), so a single
  copy stages the output per super-tile.
- Each pair's whole 5-stage layer chain lives in ONE rotating PSUM bank: a
  stage's matmul already waits on the previous stage's relu (data dep), so
  the same-bank write-after-read costs nothing. 7 rotating banks -> up to 7
  pairs in flight, which keeps the PE busy through every relu latency.
- Relu moves alternate Act/DVE (21:19, Act also owns the output-stage copy,
  DVE the softplus arithmetic); softplus is relu(x) + poly-ln1p(exp(-|x|))
  so the ACT engine only ever needs one function-table set (no reloads).
- Both 4-chunk transposes per super-tile are single 3D-AP xbar instructions;
  input copies are paced behind the previous transpose (add_dep_helper) so
  the xbar mode-switch serialization never stalls a transpose; the
  post-transpose epilogue half is deferred one super-tile so its ops never
  head-of-line block the relu streams.

Point permutation within a 8192-pt super-tile: pt(k,p,q) = base + 2048k +
16p + q with t4 row 8q+c, col 128k+p - identical on input and output paths.
"""

import numpy as np
import ml_dtypes

N_TOTAL = 1048576
N_CORES = 8
NPC = N_TOTAL // N_CORES       # 131072 points per core
ST = 8192                      # points per super-tile
N_ST = NPC // ST

_CACHE = {}


def _pack_weights(ws0, ws1, ws2, wc0, wc1, wc2, wc3):
    """Build block-diagonal 'big' stationary matrices (see module docstring)."""
    bf16 = ml_dtypes.bfloat16
    f32 = np.float32
    ws0, ws1, ws2, wc0, wc1, wc2, wc3 = [
        np.asarray(w, f32) for w in (ws0, ws1, ws2, wc0, wc1, wc2, wc3)
    ]
    w0big = np.zeros((128, 8 * 128), f32)
    c0vbig = np.zeros((128, 8 * 128), f32)
    for j in range(8):
        for q, off in ((2 * j, 0), (2 * j + 1, 64)):
            w0big[8 * q: 8 * q + 3, 128 * j + off: 128 * j + off + 64] = ws0
            c0vbig[8 * q + 3: 8 * q + 6, 128 * j + off: 128 * j + off + 64] = wc0[0:3]

    def blockdiag(m):
        b = np.zeros((128, 128), f32)
        b[0:64, 0:64] = m
        b[64:128, 64:128] = m
        return b

    s1big = blockdiag(ws1)
    c0mbig = blockdiag(ws2[:, 1:16] @ wc0[3:18])  # geo folded into c0
    c1big = blockdiag(wc1)
    c2big = blockdiag(wc2)
    # Final layer: pair j scatters rgb into rows 8j+{0..2}, 8j+{4..6} and
    # sigma_raw into rows 8j+3, 8j+7 of the shared [64,512] bank (= row 4q+c').
    c3big = np.zeros((128, 8 * 64), f32)
    sigbig = np.zeros((128, 8 * 64), f32)
    for j in range(8):
        base = 64 * j + 8 * j
        c3big[0:64, base: base + 3] = wc3[:, 0:3]
        c3big[64:128, base + 4: base + 7] = wc3[:, 0:3]
        sigbig[0:64, base + 3] = ws2[:, 0]
        sigbig[64:128, base + 7] = ws2[:, 0]
    # single packed tensor; first-needed weights (w0big, s1big) lead so a
    # small head DMA unblocks the first matmuls while the rest transfers
    wpack = np.concatenate(
        [w0big, s1big, c0vbig, c0mbig, c1big, c2big, c3big, sigbig],
        axis=1)
    return {"wpack": wpack.astype(bf16)}


def _build(npts):
    import concourse.mybir as mybir
    from concourse import bacc, tile
    from concourse.tile import add_dep_helper

    dt = mybir.dt
    f32, bf16 = dt.float32, dt.bfloat16
    AF = mybir.ActivationFunctionType

    nc = bacc.Bacc()
    x_in = nc.dram_tensor("x", [npts, 6], f32, kind="ExternalInput")
    out = nc.dram_tensor("out", [npts, 4], f32, kind="ExternalOutput")
    wcols = [("w0big", 1024), ("s1big", 128), ("c0vbig", 1024),
             ("c0mbig", 128), ("c1big", 128), ("c2big", 128),
             ("c3big", 512), ("sigbig", 512)]
    wtot = sum(c for _, c in wcols)
    wdr = nc.dram_tensor("wpack", [128, wtot], bf16, kind="ExternalInput")

    n_st = npts // ST
    # x viewed per super-tile: partition p, free (k chunk, q slot, c channel)
    xv = x_in.rearrange("(s k p q) c -> s p k (q c)", k=4, p=128, q=16)
    ov = out.rearrange("(s k p q) c -> s p k (q c)", k=4, p=128, q=16)

    with tile.TileContext(nc) as tc:
        with (
            tc.tile_pool(name="const", bufs=1) as constp,
            tc.tile_pool(name="xin", bufs=4) as xinp,
            tc.tile_pool(name="t4", bufs=4) as t4p,
            tc.tile_pool(name="act", bufs=5) as actp,
            tc.tile_pool(name="h1", bufs=10) as h1p,
            tc.tile_pool(name="ost", bufs=3) as ostp,
            tc.tile_pool(name="opm", bufs=3) as opmp,
            tc.tile_pool(name="psL", bufs=7, space="PSUM") as psLp,
            tc.tile_pool(name="psC", bufs=1, space="PSUM") as psCp,
        ):
            wt = constp.tile([128, wtot], bf16, tag="wpack")
            W = {}
            off = 0
            for name, cols in wcols:
                W[name] = wt[:, off: off + cols]
                off += cols

            # three persistent repack buffers (rotate across super-tiles);
            # pad slots (q,6) (q,7) zeroed once so zero weight rows see
            # finite data
            x8s = []
            for i in range(3):
                t = constp.tile([128, 512], bf16, tag=f"x8_{i}")
                nc.vector.memset(t[:], 0.0)
                x8s.append(t)

            # relu engine pattern per super-tile: adjacent stages alternate
            # Act/DVE; 21:19 split because DVE carries the sigma epilogue
            RELU_PAT = "ADADA" * 5 + "DADAD" * 3
            rcnt = [0]

            def relu(dst, src):
                if RELU_PAT[rcnt[0] % 40] == "A":
                    nc.scalar.activation(dst, src, AF.Relu)
                else:
                    nc.vector.tensor_scalar_max(dst, src, 0.0)
                rcnt[0] += 1

            prev_tin = [None]

            def input_path(s):
                xin = xinp.tile([128, 384], f32, tag="xin")
                xi = nc.sync.dma_start(
                    xin.rearrange("p (k m) -> p k m", k=4), xv[s])
                if prev_tin[0] is not None:
                    # keep future input copies from being scheduler-hoisted
                    # ahead of earlier transposes: the xbar mode-switch
                    # serialization would make those transposes wait on them
                    add_dep_helper(xi.ins, prev_tin[0].ins,
                                   reason="xbar: xin copy after previous tin")
                x8 = x8s[s % 3]
                nc.gpsimd.tensor_copy(
                    x8.rearrange("p (k q c) -> p k q c", k=4, c=8)[:, :, :, 0:6],
                    xin.rearrange("p (k q c) -> p k q c", k=4, c=6),
                )
                t4 = t4p.tile([128, 512], bf16, tag="t4")
                # 3D out AP = 4 independent 128x128 chunk transposes in ONE
                # xbar instruction (one HWDGE pass instead of 4 serialized)
                prev_tin[0] = nc.sync.dma_start_transpose(
                    t4.rearrange("r (k p) -> r k p", k=4), x8[:])
                return t4

            def epilogue_post(s, opm, fast=False):
                # fast=True for the final super-tile: it is pure tail latency
                # after the compute drain, so run on the faster DVE/SP engines
                ew = nc.vector if fast else nc.gpsimd
                of32 = opmp.tile([128, 256], f32, tag="of32")
                ew.tensor_copy(of32[:], opm[:])
                # sigma slots carry raw values; softplus as
                # relu(x) + ln1p(exp(-|x|)) with ln1p a degree-6 polynomial
                # (max err 3e-6 on [0,1]) so the ACT engine only ever needs
                # Relu/Copy/Exp - all in one table set, zero table reloads.
                opm3 = opm.rearrange("p (m c) -> p m c", c=4)[:, :, 3]
                of3 = of32.rearrange("p (m c) -> p m c", c=4)[:, :, 3]
                spm = opmp.tile([128, 64], f32, tag="spm")
                spu = opmp.tile([128, 64], f32, tag="spu")
                spt = opmp.tile([128, 64], f32, tag="spt")
                # arithmetic stays on DVE: TensorScalar/ScalarTensorTensor
                # opcodes are not legal on the Pool (GpSimd) engine.
                # ln1p(u) ~ u(a1+u(a2+u(a3+u*a4))), max err 1.3e-4 on [0,1];
                # final step fuses relu(x)+P via (x max 0) add P
                ALU = mybir.AluOpType
                nc.vector.scalar_tensor_tensor(
                    spm[:], opm3, -1.0, opm3, op0=ALU.mult, op1=ALU.min)
                nc.scalar.activation(spu[:], spm[:], AF.Exp)
                LN1P = [0.99712544, -0.47001579, 0.22433453, -0.0584286]
                nc.vector.tensor_scalar_mul(spt[:], spu[:], LN1P[3])
                for a in (LN1P[2], LN1P[1], LN1P[0]):
                    nc.vector.scalar_tensor_tensor(
                        spt[:], spt[:], a, spu[:], op0=ALU.add, op1=ALU.mult)
                nc.vector.scalar_tensor_tensor(
                    of3, opm3, 0.0, spt[:], op0=ALU.max, op1=ALU.add)
                (nc.sync if fast else nc.gpsimd).dma_start(
                    ov[s], of32.rearrange("p (k m) -> p k m", k=4))

            # input prefetched two super-tiles ahead so the in-order SP/Pool
            # queues never trap it behind epilogue work
            from collections import deque
            pending_post = deque()
            t4q = deque([input_path(0)])
            # weights follow the first input path so xin(0) heads the DMA
            # queue and tin(0)'s xbar mode-switch wait clears early; the
            # small head DMA covers the first matmuls' weights
            nc.scalar.dma_start(wt[:, 0:1152], wdr[:, 0:1152])
            wb = nc.scalar.dma_start(wt[:, 1152:wtot], wdr[:, 1152:wtot])
            add_dep_helper(wb.ins, prev_tin[0].ins,
                           reason="xbar: bulk weights after first tin")
            t4q.append(input_path(1))
            for s in range(n_st):
                if s == n_st - 1 and pending_post:
                    # last body: run the previous tile's post first so the
                    # final tout isn't xbar-blocked by its out-copy
                    epilogue_post(*pending_post.popleft())
                if s + 2 < n_st:
                    t4q.append(input_path(s + 2))
                t4 = t4q.popleft()
                C3 = psCp.tile([64, 512], f32, tag="C3")
                for j in range(8):
                    # one PSUM bank per pair: every stage's matmul waits on the
                    # previous stage's relu anyway, so in-place reuse is free
                    P = psLp.tile([128, 512], f32, tag="L")

                    nc.tensor.matmul(P[:], W["w0big"][:, 128 * j: 128 * (j + 1)],
                                     t4[:], start=True, stop=True)
                    h0 = actp.tile([128, 512], bf16, tag="h0")
                    relu(h0[:], P[:])

                    nc.tensor.matmul(P[:], W["s1big"][:], h0[:], start=True, stop=True)
                    h1 = h1p.tile([128, 512], bf16, tag="h1")
                    relu(h1[:], P[:])

                    nc.tensor.matmul(P[:], W["c0vbig"][:, 128 * j: 128 * (j + 1)],
                                     t4[:], start=True, stop=False)
                    nc.tensor.matmul(P[:], W["c0mbig"][:], h1[:], start=False, stop=True)
                    c0 = actp.tile([128, 512], bf16, tag="c0")
                    relu(c0[:], P[:])

                    nc.tensor.matmul(P[:], W["c1big"][:], c0[:], start=True, stop=True)
                    c1h = actp.tile([128, 512], bf16, tag="c1h")
                    relu(c1h[:], P[:])

                    nc.tensor.matmul(P[:], W["c2big"][:], c1h[:], start=True, stop=True)
                    c2h = actp.tile([128, 512], bf16, tag="c2h")
                    relu(c2h[:], P[:])

                    nc.tensor.matmul(C3[:], W["c3big"][:, 64 * j: 64 * (j + 1)],
                                     c2h[:], start=(j == 0), stop=False)
                    nc.tensor.matmul(C3[:], W["sigbig"][:, 64 * j: 64 * (j + 1)],
                                     h1[:], start=False, stop=(j == 7))

                O = ostp.tile([64, 512], bf16, tag="O")
                nc.scalar.activation(O[:], C3[:], AF.Copy)

                opm = opmp.tile([128, 256], bf16, tag="opm")
                nc.sync.dma_start_transpose(
                    opm.rearrange("p (k r) -> p k r", k=4), O[:])
                # the post-transpose half of the epilogue is deferred one
                # super-tile (emitted as epilogue_post(s-1) during body s):
                # when those ops reach their engine-queue heads the xbar
                # round-trip is long finished, so nothing head-of-line
                # blocks the relu streams
                if s == n_st - 1:
                    pending_post.append((s, opm))
                    epilogue_post(*pending_post.popleft(), fast=True)
                else:
                    pending_post.append((s, opm))
                    if s > 0:
                        epilogue_post(*pending_post.popleft())
    nc.compile()
    return nc


def _run(inputs, npts=NPC, trace=False, cores=N_CORES):
    from concourse import bass_utils

    key = npts
    if key not in _CACHE:
        _CACHE[key] = _build(npts)
    nc = _CACHE[key]
    wm = _pack_weights(inputs["ws0"], inputs["ws1"], inputs["ws2"],
                       inputs["wc0"], inputs["wc1"], inputs["wc2"], inputs["wc3"])
    x = np.ascontiguousarray(np.asarray(inputs["x"], np.float32))
    xs = x.reshape(cores, npts, 6)
    in_maps = [dict(wm, x=np.ascontiguousarray(xs[c])) for c in range(cores)]
    res = bass_utils.run_bass_kernel_spmd(
        nc, in_maps, core_ids=list(range(cores)), trace=trace)
    outs = np.concatenate([r["out"] for r in res.results], axis=0)
    return outs, res


def kernel(**inputs):
    out, _ = _run(inputs)
    return out.astype(np.float32)


# revision 9
# speedup vs baseline: 1.0078x; 1.0063x over previous
"""NeRF-small MLP Bass kernel for Trainium2, 8-core data parallel.

Hidden-on-partitions, points-on-free-dim layout: input x[N,6] is loaded
point-major, repacked 6->8 slots to bf16, and xbar-transposed to
t4[128,512] where row 8q+c = channel c of 512-point slab q. All layers run
as K<=128, M<=128, N=512 matmuls with block-diagonal weights processing two
slabs (one "pair") per pass.

- geo_feat is linear in h1 and only feeds c0, so ws2[:,1:]@wc0[3:] is folded
  into one 64x64 matrix on the host: 8 matmuls and 5 PSUM->SBUF relu moves
  per 1024 points.
- rgb + sigma accumulate across all 8 pairs of a super-tile into one [64,512]
  PSUM bank whose rows are already output-ordered (row 4q+c'), so a single
  copy stages the output per super-tile.
- Each pair's whole 5-stage layer chain lives in ONE rotating PSUM bank: a
  stage's matmul already waits on the previous stage's relu (data dep), so
  the same-bank write-after-read costs nothing. 7 rotating banks -> up to 7
  pairs in flight, which keeps the PE busy through every relu latency.
- Relu moves alternate Act/DVE (21:19, Act also owns the output-stage copy,
  DVE the softplus arithmetic); softplus is relu(x) + poly-ln1p(exp(-|x|))
  so the ACT engine only ever needs one function-table set (no reloads).
- Both 4-chunk transposes per super-tile are single 3D-AP xbar instructions;
  input copies are paced behind the previous transpose (add_dep_helper) so
  the xbar mode-switch serialization never stalls a transpose; the
  post-transpose epilogue half is deferred one super-tile so its ops never
  head-of-line block the relu streams.

Point permutation within a 8192-pt super-tile: pt(k,p,q) = base + 2048k +
16p + q with t4 row 8q+c, col 128k+p - identical on input and output paths.
"""

import numpy as np
import ml_dtypes

N_TOTAL = 1048576
N_CORES = 8
NPC = N_TOTAL // N_CORES       # 131072 points per core
ST = 8192                      # points per super-tile
N_ST = NPC // ST

_CACHE = {}


def _pack_weights(ws0, ws1, ws2, wc0, wc1, wc2, wc3):
    """Build block-diagonal 'big' stationary matrices (see module docstring)."""
    bf16 = ml_dtypes.bfloat16
    f32 = np.float32
    ws0, ws1, ws2, wc0, wc1, wc2, wc3 = [
        np.asarray(w, f32) for w in (ws0, ws1, ws2, wc0, wc1, wc2, wc3)
    ]
    w0big = np.zeros((128, 8 * 128), f32)
    c0vbig = np.zeros((128, 8 * 128), f32)
    for j in range(8):
        for q, off in ((2 * j, 0), (2 * j + 1, 64)):
            w0big[8 * q: 8 * q + 3, 128 * j + off: 128 * j + off + 64] = ws0
            c0vbig[8 * q + 3: 8 * q + 6, 128 * j + off: 128 * j + off + 64] = wc0[0:3]

    def blockdiag(m):
        b = np.zeros((128, 128), f32)
        b[0:64, 0:64] = m
        b[64:128, 64:128] = m
        return b

    s1big = blockdiag(ws1)
    c0mbig = blockdiag(ws2[:, 1:16] @ wc0[3:18])  # geo folded into c0
    c1big = blockdiag(wc1)
    c2big = blockdiag(wc2)
    # Final layer: pair j scatters rgb into rows 8j+{0..2}, 8j+{4..6} and
    # sigma_raw into rows 8j+3, 8j+7 of the shared [64,512] bank (= row 4q+c').
    c3big = np.zeros((128, 8 * 64), f32)
    sigbig = np.zeros((128, 8 * 64), f32)
    for j in range(8):
        base = 64 * j + 8 * j
        c3big[0:64, base: base + 3] = wc3[:, 0:3]
        c3big[64:128, base + 4: base + 7] = wc3[:, 0:3]
        sigbig[0:64, base + 3] = ws2[:, 0]
        sigbig[64:128, base + 7] = ws2[:, 0]
    # identity for the last super-tile's PE-side output transpose
    ident = np.zeros((128, 64), f32)
    ident[0:64, 0:64] = np.eye(64, dtype=f32)
    # single packed tensor; first-needed weights (w0big, s1big) lead so a
    # small head DMA unblocks the first matmuls while the rest transfers
    wpack = np.concatenate(
        [w0big, s1big, c0vbig, c0mbig, c1big, c2big, c3big, sigbig, ident],
        axis=1)
    return {"wpack": wpack.astype(bf16)}


def _build(npts):
    import concourse.mybir as mybir
    from concourse import bacc, tile
    from concourse.tile import add_dep_helper

    dt = mybir.dt
    f32, bf16 = dt.float32, dt.bfloat16
    AF = mybir.ActivationFunctionType

    nc = bacc.Bacc()
    x_in = nc.dram_tensor("x", [npts, 6], f32, kind="ExternalInput")
    out = nc.dram_tensor("out", [npts, 4], f32, kind="ExternalOutput")
    wcols = [("w0big", 1024), ("s1big", 128), ("c0vbig", 1024),
             ("c0mbig", 128), ("c1big", 128), ("c2big", 128),
             ("c3big", 512), ("sigbig", 512), ("ident", 64)]
    wtot = sum(c for _, c in wcols)
    wdr = nc.dram_tensor("wpack", [128, wtot], bf16, kind="ExternalInput")

    n_st = npts // ST
    # x viewed per super-tile: partition p, free (k chunk, q slot, c channel)
    xv = x_in.rearrange("(s k p q) c -> s p k (q c)", k=4, p=128, q=16)
    ov = out.rearrange("(s k p q) c -> s p k (q c)", k=4, p=128, q=16)

    with tile.TileContext(nc) as tc:
        with (
            tc.tile_pool(name="const", bufs=1) as constp,
            tc.tile_pool(name="xin", bufs=4) as xinp,
            tc.tile_pool(name="t4", bufs=4) as t4p,
            tc.tile_pool(name="act", bufs=5) as actp,
            tc.tile_pool(name="h1", bufs=10) as h1p,
            tc.tile_pool(name="ost", bufs=3) as ostp,
            tc.tile_pool(name="opm", bufs=3) as opmp,
            tc.tile_pool(name="psL", bufs=7, space="PSUM") as psLp,
            tc.tile_pool(name="psC", bufs=1, space="PSUM") as psCp,
        ):
            wt = constp.tile([128, wtot], bf16, tag="wpack")
            W = {}
            off = 0
            for name, cols in wcols:
                W[name] = wt[:, off: off + cols]
                off += cols

            # three persistent repack buffers (rotate across super-tiles);
            # pad slots (q,6) (q,7) zeroed once so zero weight rows see
            # finite data
            x8s = []
            for i in range(3):
                t = constp.tile([128, 512], bf16, tag=f"x8_{i}")
                nc.vector.memset(t[:], 0.0)
                x8s.append(t)

            # relu engine pattern per super-tile: adjacent stages alternate
            # Act/DVE; 21:19 split because DVE carries the sigma epilogue
            RELU_PAT = "ADADA" * 5 + "DADAD" * 3
            rcnt = [0]

            def relu(dst, src):
                if RELU_PAT[rcnt[0] % 40] == "A":
                    nc.scalar.activation(dst, src, AF.Relu)
                else:
                    nc.vector.tensor_scalar_max(dst, src, 0.0)
                rcnt[0] += 1

            # PE clock pre-warm: the input chain keeps the PE idle for the
            # first ~7us, so its first real matmuls would run at the cold/mid
            # p-state. Dummy matmuls on a zeroed scratch tile during that
            # window warm the clock gate for free (never read; harmless).
            warm_src = constp.tile([128, 128], bf16, tag="warm")
            nc.vector.memset(warm_src[:], 0.0)
            warm_ps = psCp.tile([128, 128], f32, tag="C3")
            for _ in range(28):
                nc.tensor.matmul(warm_ps[:], warm_src[:], warm_src[:],
                                 start=True, stop=True)

            prev_tin = [None]

            def input_path(s):
                xin = xinp.tile([128, 384], f32, tag="xin")
                xi = nc.sync.dma_start(
                    xin.rearrange("p (k m) -> p k m", k=4), xv[s])
                if prev_tin[0] is not None:
                    # keep future input copies from being scheduler-hoisted
                    # ahead of earlier transposes: the xbar mode-switch
                    # serialization would make those transposes wait on them
                    add_dep_helper(xi.ins, prev_tin[0].ins,
                                   reason="xbar: xin copy after previous tin")
                x8 = x8s[s % 3]
                nc.gpsimd.tensor_copy(
                    x8.rearrange("p (k q c) -> p k q c", k=4, c=8)[:, :, :, 0:6],
                    xin.rearrange("p (k q c) -> p k q c", k=4, c=6),
                )
                t4 = t4p.tile([128, 512], bf16, tag="t4")
                # 3D out AP = 4 independent 128x128 chunk transposes in ONE
                # xbar instruction (one HWDGE pass instead of 4 serialized)
                prev_tin[0] = nc.sync.dma_start_transpose(
                    t4.rearrange("r (k p) -> r k p", k=4), x8[:])
                return t4

            def epilogue_post(s, opm, fast=False):
                # fast=True for the final super-tile: it is pure tail latency
                # after the compute drain, so run on the faster DVE/SP engines
                ew = nc.vector if fast else nc.gpsimd
                of32 = opmp.tile([128, 256], f32, tag="of32")
                ew.tensor_copy(of32[:], opm[:])
                # sigma slots carry raw values; softplus as
                # relu(x) + ln1p(exp(-|x|)) with ln1p a degree-6 polynomial
                # (max err 3e-6 on [0,1]) so the ACT engine only ever needs
                # Relu/Copy/Exp - all in one table set, zero table reloads.
                opm3 = opm.rearrange("p (m c) -> p m c", c=4)[:, :, 3]
                of3 = of32.rearrange("p (m c) -> p m c", c=4)[:, :, 3]
                spm = opmp.tile([128, 64], f32, tag="spm")
                spu = opmp.tile([128, 64], f32, tag="spu")
                spt = opmp.tile([128, 64], f32, tag="spt")
                # arithmetic stays on DVE: TensorScalar/ScalarTensorTensor
                # opcodes are not legal on the Pool (GpSimd) engine.
                # ln1p(u) ~ u(a1+u(a2+u(a3+u*a4))), max err 1.3e-4 on [0,1];
                # final step fuses relu(x)+P via (x max 0) add P
                ALU = mybir.AluOpType
                if fast:
                    # opm lives in PSUM here; vector ops may read only one
                    # PSUM operand, so build exp(-|x|) via two ACT ops
                    nc.scalar.activation(spm[:], opm3, AF.Abs)
                    nc.scalar.activation(spu[:], spm[:], AF.Exp, scale=-1.0)
                else:
                    nc.vector.scalar_tensor_tensor(
                        spm[:], opm3, -1.0, opm3, op0=ALU.mult, op1=ALU.min)
                    nc.scalar.activation(spu[:], spm[:], AF.Exp)
                LN1P = [0.99712544, -0.47001579, 0.22433453, -0.0584286]
                nc.vector.tensor_scalar_mul(spt[:], spu[:], LN1P[3])
                for a in (LN1P[2], LN1P[1], LN1P[0]):
                    nc.vector.scalar_tensor_tensor(
                        spt[:], spt[:], a, spu[:], op0=ALU.add, op1=ALU.mult)
                nc.vector.scalar_tensor_tensor(
                    of3, opm3, 0.0, spt[:], op0=ALU.max, op1=ALU.add)
                (nc.sync if fast else nc.gpsimd).dma_start(
                    ov[s], of32.rearrange("p (k m) -> p k m", k=4))

            # input prefetched two super-tiles ahead so the in-order SP/Pool
            # queues never trap it behind epilogue work
            from collections import deque
            pending_post = deque()
            t4q = deque([input_path(0)])
            # weights follow the first input path so xin(0) heads the DMA
            # queue and tin(0)'s xbar mode-switch wait clears early; the
            # small head DMA covers the first matmuls' weights
            nc.scalar.dma_start(wt[:, 0:1152], wdr[:, 0:1152])
            wb = nc.scalar.dma_start(wt[:, 1152:wtot], wdr[:, 1152:wtot])
            add_dep_helper(wb.ins, prev_tin[0].ins,
                           reason="xbar: bulk weights after first tin")
            t4q.append(input_path(1))
            for s in range(n_st):
                if s == n_st - 1 and pending_post:
                    # last body: run the previous tile's post first so the
                    # final tout isn't xbar-blocked by its out-copy
                    epilogue_post(*pending_post.popleft())
                if s + 2 < n_st:
                    t4q.append(input_path(s + 2))
                t4 = t4q.popleft()
                C3 = psCp.tile([64, 512], f32, tag="C3")
                for j in range(8):
                    # one PSUM bank per pair: every stage's matmul waits on the
                    # previous stage's relu anyway, so in-place reuse is free
                    P = psLp.tile([128, 512], f32, tag="L")

                    nc.tensor.matmul(P[:], W["w0big"][:, 128 * j: 128 * (j + 1)],
                                     t4[:], start=True, stop=True)
                    h0 = actp.tile([128, 512], bf16, tag="h0")
                    relu(h0[:], P[:])

                    nc.tensor.matmul(P[:], W["s1big"][:], h0[:], start=True, stop=True)
                    h1 = h1p.tile([128, 512], bf16, tag="h1")
                    relu(h1[:], P[:])

                    nc.tensor.matmul(P[:], W["c0vbig"][:, 128 * j: 128 * (j + 1)],
                                     t4[:], start=True, stop=False)
                    nc.tensor.matmul(P[:], W["c0mbig"][:], h1[:], start=False, stop=True)
                    c0 = actp.tile([128, 512], bf16, tag="c0")
                    relu(c0[:], P[:])

                    nc.tensor.matmul(P[:], W["c1big"][:], c0[:], start=True, stop=True)
                    c1h = actp.tile([128, 512], bf16, tag="c1h")
                    relu(c1h[:], P[:])

                    nc.tensor.matmul(P[:], W["c2big"][:], c1h[:], start=True, stop=True)
                    c2h = actp.tile([128, 512], bf16, tag="c2h")
                    relu(c2h[:], P[:])

                    nc.tensor.matmul(C3[:], W["c3big"][:, 64 * j: 64 * (j + 1)],
                                     c2h[:], start=(j == 0), stop=False)
                    nc.tensor.matmul(C3[:], W["sigbig"][:, 64 * j: 64 * (j + 1)],
                                     h1[:], start=False, stop=(j == 7))

                O = ostp.tile([64, 512], bf16, tag="O")
                nc.scalar.activation(O[:], C3[:], AF.Copy)

                if s == n_st - 1:
                    # tail: PE is idle once compute drains, so transpose the
                    # final output on the PE (~0.3us) instead of paying the
                    # ~2.4us xbar round-trip after the last matmul
                    opm = psLp.tile([128, 256], bf16, tag="L")
                    for k in range(4):
                        nc.tensor.transpose(
                            opm[:, 64 * k: 64 * (k + 1)],
                            O[:, 128 * k: 128 * (k + 1)],
                            W["ident"][0:64, 0:64])
                else:
                    opm = opmp.tile([128, 256], bf16, tag="opm")
                    nc.sync.dma_start_transpose(
                        opm.rearrange("p (k r) -> p k r", k=4), O[:])
                # the post-transpose half of the epilogue is deferred one
                # super-tile (emitted as epilogue_post(s-1) during body s):
                # when those ops reach their engine-queue heads the xbar
                # round-trip is long finished, so nothing head-of-line
                # blocks the relu streams
                if s == n_st - 1:
                    pending_post.append((s, opm))
                    epilogue_post(*pending_post.popleft(), fast=True)
                else:
                    pending_post.append((s, opm))
                    if s > 0:
                        epilogue_post(*pending_post.popleft())
    nc.compile()
    return nc


def _run(inputs, npts=NPC, trace=False, cores=N_CORES):
    from concourse import bass_utils

    key = npts
    if key not in _CACHE:
        _CACHE[key] = _build(npts)
    nc = _CACHE[key]
    wm = _pack_weights(inputs["ws0"], inputs["ws1"], inputs["ws2"],
                       inputs["wc0"], inputs["wc1"], inputs["wc2"], inputs["wc3"])
    x = np.ascontiguousarray(np.asarray(inputs["x"], np.float32))
    xs = x.reshape(cores, npts, 6)
    in_maps = [dict(wm, x=np.ascontiguousarray(xs[c])) for c in range(cores)]
    res = bass_utils.run_bass_kernel_spmd(
        nc, in_maps, core_ids=list(range(cores)), trace=trace)
    outs = np.concatenate([r["out"] for r in res.results], axis=0)
    return outs, res


def kernel(**inputs):
    out, _ = _run(inputs)
    return out.astype(np.float32)
